# revision 28
# baseline (speedup 1.0000x reference)
"""Multi-head linear attention on 8 Trainium2 NeuronCores.

Sharding: data-parallel over batch (4) x tensor-parallel over heads (2 groups
of 8). Core c handles batch c//2, head-group c%2. Each core computes its
head-group's partial output projection; the host sums the two partials per
batch.

All matmul operands are bf16 (f32 PSUM accumulation); inputs/weights are cast
to bf16 on the host and laid out tile-contiguously so every tile is one DMA
with 2KB+ lines. Per-core math (F=1024, L=8192, HG=8 heads, D=64, HD=512):

  phase 1 (keys/values, per 128-token tile):
    k = xkv @ Wk ; v = xkv @ Wv          ([tok, hd] in PSUM)
    kp = phi(k) = exp(min(k,0)) + max(k,0)   (bf16)
    st_j += kp_j^T @ [v_j | 1]           (PSUM state per head pair: kv + ksum)
  transition:
    kv2 = blockdiag(S_h) ; blk = blockdiag(ksum_h)
    kv2T = PE-transpose(kv2) ; G = kv2T^T @ Wo = S @ Wo   (folds Wo into state)
  phase 2 (queries, per 512-token tile, software-pipelined one stage):
    qT = Wq^T @ xq^T ; qp = phi(q)       ([hd, tok])
    den = blk^T @ qp ; z = 1/(den+eps)
    zb = E^T z (broadcast z over d) ; qp' = qp * zb
    yT += G^T @ qp'                      ([f, tok] partial, summed on host)
"""

import sys

sys.path.insert(0, "/opt/trn_rl_repo")

import numpy as np
import ml_dtypes

import concourse.bass as bass  # noqa: F401  (import keeps bass registered)
import concourse.tile as tile
from concourse import bacc, mybir
from concourse.bass_utils import run_bass_kernel_spmd

F32 = mybir.dt.float32
F32R = mybir.dt.float32r
BF16 = mybir.dt.bfloat16
AF = mybir.ActivationFunctionType
ALU = mybir.AluOpType

B, L_FULL, F = 4, 8192, 1024
H, D = 16, 64
N_CORES = 8
HG = H // 2  # heads per core = 8
HD = HG * D  # 512
EPS = 1e-6


def build_nc(L=L_FULL, TQ=512, TK=128):
    NKT = L // TK
    NQT = L // TQ
    FA = F // 128  # 8 f-tiles
    NM = HD // 128  # 4 hd-tiles

    nc = bacc.Bacc("TRN2", target_bir_lowering=False, debug=False)

    # inputs pre-tiled on host: [partition, tile, f-chunk, token]
    xqT = nc.dram_tensor("xqT", [128, NQT, FA, TQ], BF16, kind="ExternalInput")
    xkvT = nc.dram_tensor("xkvT", [128, NKT, FA, TK], BF16, kind="ExternalInput")
    wq = nc.dram_tensor("wq", [F, HD], BF16, kind="ExternalInput")
    wk = nc.dram_tensor("wk", [F, HD], BF16, kind="ExternalInput")
    wv = nc.dram_tensor("wv", [F, HD], BF16, kind="ExternalInput")
    wo = nc.dram_tensor("wo", [HD, F], BF16, kind="ExternalInput")
    em = nc.dram_tensor("ematrix", [HG, NM, 128], BF16, kind="ExternalInput")
    idm = nc.dram_tensor("ident", [128, 128], F32R, kind="ExternalInput")
    yT = nc.dram_tensor("yT", [128, NQT, FA, TQ], BF16, kind="ExternalOutput")

    wq_r = wq.rearrange("(a p) n -> p a n", p=128)
    wk_r = wk.rearrange("(a p) n -> p a n", p=128)
    wv_r = wv.rearrange("(a p) n -> p a n", p=128)
    wo_r = wo.rearrange("(m p) f -> p m f", p=128)

    with tile.TileContext(nc) as tc:
        with (
            tc.tile_pool(name="singles", bufs=1) as singles,
            tc.tile_pool(name="kv_in", bufs=5) as kv_in,
            tc.tile_pool(name="kwork", bufs=2) as kwork,
            tc.tile_pool(name="q_in", bufs=3) as q_in,
            tc.tile_pool(name="qwork", bufs=2) as qwork,
            tc.tile_pool(name="yout", bufs=2) as yout,
        ):
            # ps_kv first so ps_state (exited at the transition, under LIFO)
            # sits above it; ps_q then lands on the freed pk/pv banks whose
            # readers finish immediately, not on the state banks.
            ps_kv_ctx = tc.tile_pool(name="ps_kv", bufs=2, space="PSUM")
            ps_kv = ps_kv_ctx.__enter__()
            ps_state_ctx = tc.tile_pool(name="ps_state", bufs=1, space="PSUM")
            ps_state = ps_state_ctx.__enter__()
            # ---- weights on the scalar + gpsimd rings (sync ring stays a
            # pure xkv/xq/y stream); first wk chunk small so the first k
            # matmul can start ~10us in
            wq_sb = singles.tile([128, FA, HD], BF16)
            wk_sb = singles.tile([128, FA, HD], BF16)
            wv_sb = singles.tile([128, FA, HD], BF16)
            wo_sb = singles.tile([128, NM, F], BF16)
            nc.scalar.dma_start(out=wk_sb[:, 0:1, :], in_=wk_r[:, 0:1, :])
            nc.scalar.dma_start(out=wk_sb[:, 1:2, :], in_=wk_r[:, 1:2, :])
            nc.scalar.dma_start(out=wk_sb[:, 2:4, :], in_=wk_r[:, 2:4, :])
            nc.scalar.dma_start(out=wk_sb[:, 4:6, :], in_=wk_r[:, 4:6, :])
            nc.scalar.dma_start(out=wk_sb[:, 6:8, :], in_=wk_r[:, 6:8, :])
            # persistent bf16 ones for the ksum column of the state matmul
            ones_sb = singles.tile([128, NM, 1], BF16)
            nc.vector.memset(ones_sb[:], 1.0)
            # broadcast matrix E (E[2j, j, 0:64]=1, E[2j+1, j, 64:128]=1) and
            # 128x128 identity for the PE transpose; DMA'd mid phase 1
            e_sb = singles.tile([HG, NM, 128], BF16)
            id_sb = singles.tile([128, 128], F32R)

            # persistent state accumulators, one PSUM bank per head pair j.
            # Single matmul per pair: lhsT = kp[:, pair d-range] (128 wide),
            # rhs = [v_h0 | v_h1 | 1] (129 wide). Rows 0:64 x cols 0:64 give
            # head 2j's kv, rows 64:128 x cols 64:128 head 2j+1's kv (cross
            # blocks never read); col 128 is the stacked ksum pair.
            st_ps = [
                ps_state.tile([128, 2 * D + 1], F32, tag=f"st{j}", name=f"st_ps{j}")
                for j in range(NM)
            ]

            # ---- phase 1: keys/values ----
            # Software pipeline, k-side LAG ahead of v-side: the first k
            # projections only need Wk; the static PE instruction order must
            # have them first or the PE stalls on the v-side wait at startup.
            LAG = 3
            xq_pre = {}
            xkv_tiles = {}
            kp_tiles = {}

            def k_side(kt):
                xkv_t = kv_in.tile([128, FA, TK], BF16, tag="xkv", name=f"xkv{kt}")
                nc.sync.dma_start(out=xkv_t[:], in_=xkvT[:, kt, :, :])
                xkv_tiles[kt] = xkv_t
                pk = ps_kv.tile([128, HD], F32, tag="pk", name=f"pk{kt}")
                for a in range(FA):
                    nc.tensor.matmul(
                        pk[:],
                        lhsT=xkv_t[:, a, :],
                        rhs=wk_sb[:, a, :],
                        start=(a == 0),
                        stop=(a == FA - 1),
                    )
                # phi(k) = exp(min(k,0)) + max(k,0); kp in bf16 (state-only)
                tmp = kwork.tile([128, HD], F32, tag="tmp", name=f"tmp{kt}")
                nc.vector.tensor_scalar_min(tmp[:], pk[:], 0.0)
                ek = kwork.tile([128, HD], F32, tag="ek", name=f"ek{kt}")
                nc.scalar.activation(ek[:], tmp[:], AF.Exp)
                kp = kwork.tile([128, HD], BF16, tag="kp", name=f"kp{kt}", bufs=3)
                nc.vector.scalar_tensor_tensor(
                    kp[:], in0=pk[:], scalar=0.0, in1=ek[:], op0=ALU.max, op1=ALU.add
                )
                kp_tiles[kt] = kp

            def v_side(kt):
                xkv_t = xkv_tiles.pop(kt)
                kp = kp_tiles.pop(kt)
                pv = ps_kv.tile([128, HD], F32, tag="pv", name=f"pv{kt}")
                for a in range(FA):
                    nc.tensor.matmul(
                        pv[:],
                        lhsT=xkv_t[:, a, :],
                        rhs=wv_sb[:, a, :],
                        start=(a == 0),
                        stop=(a == FA - 1),
                    )
                # v pairs with trailing ones column: [v_h0 | v_h1 | 1]
                v_sb = kwork.tile([128, NM, 2 * D + 1], BF16, tag="v", name=f"v{kt}")
                nc.scalar.copy(
                    out=v_sb[:, :, 0 : 2 * D],
                    in_=pv[:].rearrange("p (j w) -> p j w", j=NM),
                )
                nc.vector.tensor_copy(v_sb[:, :, 2 * D : 2 * D + 1], ones_sb[:])
                # state accumulation, one stream per head pair bank
                for j in range(NM):
                    nc.tensor.matmul(
                        st_ps[j][:],
                        lhsT=kp[:, 2 * j * D : (2 * j + 2) * D],
                        rhs=v_sb[:, j, :],
                        start=(kt == 0),
                        stop=(kt == NKT - 1),
                    )

            # prologue: k-sides of the first LAG tiles run before any v work
            # (only needs Wk); wv rides the sync ring between the first xkv
            # tiles so it lands on a fast HWDGE ring before v_side(0)
            k_side(0)
            k_side(1)
            nc.sync.dma_start(out=wv_sb[:, 0:2, :], in_=wv_r[:, 0:2, :])
            nc.sync.dma_start(out=wv_sb[:, 2:4, :], in_=wv_r[:, 2:4, :])
            k_side(2)
            nc.sync.dma_start(out=wv_sb[:, 4:6, :], in_=wv_r[:, 4:6, :])
            nc.sync.dma_start(out=wv_sb[:, 6:8, :], in_=wv_r[:, 6:8, :])
            for i in range(LAG):
                v_side(i)
            for i in range(LAG, NKT):
                if i == NKT // 2:
                    # phase-2 weights + broadcast/identity mats on idle ring
                    for c in range(4):
                        nc.gpsimd.dma_start(
                            out=wq_sb[:, 2 * c : 2 * c + 2, :],
                            in_=wq_r[:, 2 * c : 2 * c + 2, :],
                        )
                    for m in range(NM):
                        nc.gpsimd.dma_start(out=wo_sb[:, m, :], in_=wo_r[:, m, :])
                    nc.gpsimd.dma_start(out=e_sb[:], in_=em[:])
                    nc.gpsimd.dma_start(out=id_sb[:], in_=idm[:])
                if NKT > 16 and i in (NKT - 8, NKT - 4):
                    qi = 0 if i == NKT - 8 else 1
                    t_pre = q_in.tile([128, FA, TQ], BF16, tag="xq", name=f"xq_pre{qi}")
                    nc.sync.dma_start(out=t_pre[:], in_=xqT[:, qi, :, :])
                    xq_pre[qi] = t_pre
                k_side(i)
                v_side(i)

            # ---- transition part A (emitted before phase-2 round 0 so
            # vector/scalar fill kv2/blk while the PE runs q-projections):
            # block-diagonal kv pairs + block-diag ksum from the state PSUM.
            kv2 = singles.tile([128, NM, 128], F32R)
            nc.vector.memset(kv2[:].bitcast(F32), 0.0)
            blk = singles.tile([128, NM, HG], BF16)
            nc.vector.memset(blk[:], 0.0)
            for j in range(NM):
                nc.vector.tensor_copy(kv2[0:64, j, 0:D], st_ps[j][0:64, 0:D])
                nc.vector.tensor_copy(
                    kv2[64:128, j, D:128], st_ps[j][64:128, D : 2 * D]
                )
                nc.scalar.copy(
                    out=blk[0:64, j, 2 * j : 2 * j + 1],
                    in_=st_ps[j][0:64, 2 * D : 2 * D + 1],
                )
                nc.scalar.copy(
                    out=blk[64:128, j, 2 * j + 1 : 2 * j + 2],
                    in_=st_ps[j][64:128, 2 * D : 2 * D + 1],
                )
            ps_state_ctx.__exit__(None, None, None)
            ps_kv_ctx.__exit__(None, None, None)

            # phase-2 PSUM: pq/pd 3 + zb 2 + py 3 = 8 banks
            ps_q_ctx = tc.tile_pool(name="ps_q", bufs=3, space="PSUM")
            ps_q = ps_q_ctx.__enter__()

            kv2T = singles.tile([128, NM, 128], BF16)
            g_sb = singles.tile([128, NM, F], BF16)

            qp_tiles = {}
            z_tiles = {}

            def q_side(qt, emit_z, prev=None):
                # prev = (qp2, qp, z_sb) of round qt-1: its zb matmuls and
                # qp2 multiplies are interleaved into this round's m-loop so
                # the vector engine produces qp2[m] right after phi[m] and
                # the first y matmul of round qt-1 never waits on qp2.
                if qt in xq_pre:
                    xq_t = xq_pre.pop(qt)
                else:
                    xq_t = q_in.tile([128, FA, TQ], BF16, tag="xq", name=f"xq_t{qt}")
                    nc.sync.dma_start(out=xq_t[:], in_=xqT[:, qt, :, :])
                qp = qwork.tile([128, NM, TQ], BF16, tag="qp")
                for m in range(NM):
                    pq = ps_q.tile([128, TQ], F32, tag="pq")
                    for a in range(FA):
                        nc.tensor.matmul(
                            pq[:],
                            lhsT=wq_sb[:, a, m * 128 : (m + 1) * 128],
                            rhs=xq_t[:, a, :],
                            start=(a == 0),
                            stop=(a == FA - 1),
                        )
                    if prev is not None:
                        pqp2, pqp, pz = prev
                        pzb = ps_zb.tile(
                            [128, TQ], F32, tag="zb", name=f"zb{qt}_{m}"
                        )
                        nc.tensor.matmul(
                            pzb[:],
                            lhsT=e_sb[:, m, :],
                            rhs=pz[:],
                            start=True,
                            stop=True,
                        )
                    tmp2 = qwork.tile([128, TQ], F32, tag="tmp2")
                    nc.vector.tensor_scalar_min(tmp2[:], pq[:], 0.0)
                    eq = qwork.tile([128, TQ], F32, tag="eq")
                    nc.scalar.activation(eq[:], tmp2[:], AF.Exp)
                    nc.vector.scalar_tensor_tensor(
                        qp[:, m, :],
                        in0=pq[:],
                        scalar=0.0,
                        in1=eq[:],
                        op0=ALU.max,
                        op1=ALU.add,
                    )
                    if prev is not None:
                        nc.vector.tensor_mul(pqp2[:, m, :], pqp[:, m, :], pzb[:])
                qp_tiles[qt] = qp
                # denominator [HG, TQ], accumulated over m; shares the pq
                # bank ring so steady-state PSUM stays at 8 banks
                pd = ps_q.tile([HG, TQ], F32, tag="pq", name=f"pd{qt}")
                for m in range(NM):
                    nc.tensor.matmul(
                        pd[:],
                        lhsT=blk[:, m, :],
                        rhs=qp[:, m, :],
                        start=(m == 0),
                        stop=(m == NM - 1),
                    )
                if emit_z:
                    z_recip(qt, pd)
                else:
                    pd_tiles[qt] = pd

            pd_tiles = {}

            def z_recip(qt, pd):
                # z = 1/den as exp(-ln(den)) on the scalar engine: keeps the
                # reciprocal off the (near-saturated) vector engine. den is
                # a sum of ~L positive terms (>=1e3), so ln is safe and the
                # reference's +eps is numerically irrelevant.
                zs = qwork.tile([HG, TQ], F32, tag="zs")
                nc.scalar.activation(zs[:], pd[:], AF.Ln)
                z_sb = qwork.tile([HG, TQ], BF16, tag="z")
                nc.scalar.activation(z_sb[:], zs[:], AF.Exp, scale=-1.0)
                z_tiles[qt] = z_sb

            def make_qp2(qt):
                # zb + qp2 for round qt are emitted interleaved inside
                # q_side(qt+1) (or inline for the last round)
                qp = qp_tiles.pop(qt)
                z_sb = z_tiles.pop(qt)
                qp2 = qwork.tile([128, NM, TQ], BF16, tag="qp2", name=f"qp2_{qt}")
                return (qp2, qp, z_sb)

            def out_side(qt, qp2, next_qt):
                # z for the next round (pd already accumulated there)
                if next_qt is not None:
                    z_recip(next_qt, pd_tiles.pop(next_qt))
                # output projection: fo-quarters, m-outer accumulation so the
                # first y matmul only needs qp2[m=0]
                y_sb = yout.tile([128, FA, TQ], BF16)
                for q4 in range(4):
                    pys = [
                        ps_y.tile([128, TQ], F32, tag="py", name=f"py{qt}_{q4}_{w}")
                        for w in range(2)
                    ]
                    for m in range(NM):
                        for w in range(2):
                            fo = q4 * 2 + w
                            nc.tensor.matmul(
                                pys[w][:],
                                lhsT=g_sb[:, m, fo * 128 : (fo + 1) * 128],
                                rhs=qp2[:, m, :],
                                start=(m == 0),
                                stop=(m == NM - 1),
                            )
                    for w in range(2):
                        # balance PSUM->SBUF copies: 2 of 8 go to the vector
                        # engine so the scalar engine (exp + ln/exp z) keeps up
                        if w == 1 and q4 < 2:
                            nc.vector.tensor_copy(y_sb[:, q4 * 2 + w, :], pys[w][:])
                        else:
                            nc.scalar.copy(out=y_sb[:, q4 * 2 + w, :], in_=pys[w][:])
                    if qt == NQT - 1:
                        # drain the tail: ship each quarter as soon as copied
                        nc.sync.dma_start(
                            out=yT[:, qt, 2 * q4 : 2 * q4 + 2, :],
                            in_=y_sb[:, 2 * q4 : 2 * q4 + 2, :],
                        )
                if qt < NQT - 1:
                    nc.sync.dma_start(
                        out=yT[:, qt, 0 : FA // 2, :], in_=y_sb[:, 0 : FA // 2, :]
                    )
                    nc.sync.dma_start(
                        out=yT[:, qt, FA // 2 : FA, :], in_=y_sb[:, FA // 2 : FA, :]
                    )

            # round 0: q-side, then transition part B (PE transpose + G) so
            # the state->Wo fold hides under the first q-projections.
            q_side(0, emit_z=True)
            ps_tr_ctx = tc.tile_pool(name="ps_tr", bufs=1, space="PSUM")
            ps_tr = ps_tr_ctx.__enter__()
            ps_g_ctx = tc.tile_pool(name="ps_g", bufs=1, space="PSUM")
            ps_g = ps_g_ctx.__enter__()
            for j in range(NM):
                trp = ps_tr.tile([128, 128], F32R, tag="tr", name=f"tr{j}")
                nc.tensor.transpose(trp[:], kv2[:, j, :], id_sb[:])
                nc.vector.tensor_copy(kv2T[:, j, :], trp[:].bitcast(F32))
            for j in range(NM):
                for fh in range(2):
                    gp = ps_g.tile([128, F // 2], F32, tag="g", name=f"g{j}_{fh}")
                    nc.tensor.matmul(
                        gp[:],
                        lhsT=kv2T[:, j, :],
                        rhs=wo_sb[:, j, fh * (F // 2) : (fh + 1) * (F // 2)],
                        start=True,
                        stop=True,
                    )
                    nc.scalar.copy(
                        out=g_sb[:, j, fh * (F // 2) : (fh + 1) * (F // 2)], in_=gp[:]
                    )
            ps_g_ctx.__exit__(None, None, None)
            ps_tr_ctx.__exit__(None, None, None)

            ps_zb_ctx = tc.tile_pool(name="ps_zb", bufs=2, space="PSUM")
            ps_zb = ps_zb_ctx.__enter__()
            ps_y_ctx = tc.tile_pool(name="ps_y", bufs=3, space="PSUM")
            ps_y = ps_y_ctx.__enter__()

            for qt in range(1, NQT):
                prev = make_qp2(qt - 1)
                q_side(qt, emit_z=False, prev=prev)
                out_side(qt - 1, prev[0], qt)
            # last round: no next q_side to host the interleave
            lqp2, lqp, lz = make_qp2(NQT - 1)
            for m in range(NM):
                pzb = ps_zb.tile([128, TQ], F32, tag="zb", name=f"zbL_{m}")
                nc.tensor.matmul(
                    pzb[:], lhsT=e_sb[:, m, :], rhs=lz[:], start=True, stop=True
                )
                nc.vector.tensor_mul(lqp2[:, m, :], lqp[:, m, :], pzb[:])
            out_side(NQT - 1, lqp2, None)

            ps_y_ctx.__exit__(None, None, None)
            ps_zb_ctx.__exit__(None, None, None)
            ps_q_ctx.__exit__(None, None, None)

    nc.finalize()
    return nc


_NC_CACHE = {}


def _get_nc(L):
    if L not in _NC_CACHE:
        _NC_CACHE[L] = build_nc(L=L)
    return _NC_CACHE[L]


def _tile_T(x, nt, tt):
    # [L, F] f32 -> [128, nt, FA, tt] bf16 tile-contiguous transposed layout
    l_, f_ = x.shape
    fa = f_ // 128
    arr = np.ascontiguousarray(x.T).astype(ml_dtypes.bfloat16)
    return np.ascontiguousarray(
        arr.reshape(fa, 128, nt, tt).transpose(1, 2, 0, 3)
    )


def make_in_maps(inputs_q, inputs_kv, Wq, Wk, Wv, Wo, TQ=512, TK=128):
    inputs_q = np.asarray(inputs_q, dtype=np.float32)
    inputs_kv = np.asarray(inputs_kv, dtype=np.float32)
    Wq = np.asarray(Wq, dtype=np.float32)
    Wk = np.asarray(Wk, dtype=np.float32)
    Wv = np.asarray(Wv, dtype=np.float32)
    Wo = np.asarray(Wo, dtype=np.float32)
    b_ = inputs_q.shape[0]
    l_ = inputs_q.shape[1]
    nqt, nkt = l_ // TQ, l_ // TK
    xqT = [_tile_T(inputs_q[b], nqt, TQ) for b in range(b_)]
    xkvT = [_tile_T(inputs_kv[b], nkt, TK) for b in range(b_)]
    f_ = Wq.shape[0]
    BF = ml_dtypes.bfloat16
    wq_g = [
        np.ascontiguousarray(Wq[:, g * HG : (g + 1) * HG, :].reshape(f_, HD)).astype(BF)
        for g in range(2)
    ]
    wk_g = [
        np.ascontiguousarray(Wk[:, g * HG : (g + 1) * HG, :].reshape(f_, HD)).astype(BF)
        for g in range(2)
    ]
    wv_g = [
        np.ascontiguousarray(Wv[:, g * HG : (g + 1) * HG, :].reshape(f_, HD)).astype(BF)
        for g in range(2)
    ]
    wo_g = [
        np.ascontiguousarray(Wo[g * HG : (g + 1) * HG].reshape(HD, f_)).astype(BF)
        for g in range(2)
    ]
    em = make_ematrix()
    ident = np.eye(128, dtype=np.float32)
    in_maps = []
    for c in range(2 * b_):
        b, g = c // 2, c % 2
        in_maps.append(
            {
                "xqT": xqT[b],
                "xkvT": xkvT[b],
                "wq": wq_g[g],
                "wk": wk_g[g],
                "wv": wv_g[g],
                "wo": wo_g[g],
                "ematrix": em,
                "ident": ident,
            }
        )
    return in_maps


def make_ematrix():
    em = np.zeros((HG, HD // 128, 128), dtype=ml_dtypes.bfloat16)
    for j in range(HD // 128):
        em[2 * j, j, 0:64] = 1.0
        em[2 * j + 1, j, 64:128] = 1.0
    return em


def run(inputs_q, inputs_kv, Wq, Wk, Wv, Wo, trace=False, **spmd_kwargs):
    l_ = np.asarray(inputs_q).shape[1]
    TQ = 512
    nc = _get_nc(l_)
    in_maps = make_in_maps(inputs_q, inputs_kv, Wq, Wk, Wv, Wo)
    res = run_bass_kernel_spmd(
        nc, in_maps, list(range(len(in_maps))), trace=trace, **spmd_kwargs
    )
    b_ = len(in_maps) // 2
    nqt = l_ // TQ
    out = np.empty((b_, l_, F), dtype=np.float32)
    for b in range(b_):
        # yT: [128, NQT, FA, TQ] bf16 partials; f = a*128 + p, l = qt*TQ + t
        s = res.results[2 * b]["yT"].astype(np.float32) + res.results[2 * b + 1][
            "yT"
        ].astype(np.float32)
        np.copyto(out[b], s.transpose(1, 3, 2, 0).reshape(l_, F))
    return out, res


def kernel(inputs_q, inputs_kv, Wq, Wk, Wv, Wo):
    out, _ = run(inputs_q, inputs_kv, Wq, Wk, Wv, Wo)
    return out


# revision 33
# speedup vs baseline: 1.0370x; 1.0370x over previous
"""Multi-head linear attention on 8 Trainium2 NeuronCores.

Sharding: data-parallel over batch (4) x tensor-parallel over heads (2 groups
of 8). Core c handles batch c//2, head-group c%2. Each core computes its
head-group's partial output projection; the host sums the two partials per
batch.

All matmul operands are bf16 (f32 PSUM accumulation); inputs/weights are cast
to bf16 on the host and laid out tile-contiguously so every tile is one DMA
with 2KB+ lines. Per-core math (F=1024, L=8192, HG=8 heads, D=64, HD=512):

  phase 1 (keys/values, per 128-token tile):
    k = xkv @ Wk ; v = xkv @ Wv          ([tok, hd] in PSUM)
    kp = phi(k) = exp(min(k,0)) + max(k,0)   (bf16)
    st_j += kp_j^T @ [v_j | 1]           (PSUM state per head pair: kv + ksum)
  transition:
    kv2 = blockdiag(S_h) ; blk = blockdiag(ksum_h)
    kv2T = PE-transpose(kv2) ; G = kv2T^T @ Wo = S @ Wo   (folds Wo into state)
  phase 2 (queries, per 512-token tile, software-pipelined one stage):
    qT = Wq^T @ xq^T ; qp = phi(q)       ([hd, tok])
    den = blk^T @ qp ; z = 1/(den+eps)
    zb = E^T z (broadcast z over d) ; qp' = qp * zb
    yT += G^T @ qp'                      ([f, tok] partial, summed on host)
"""

import sys

sys.path.insert(0, "/opt/trn_rl_repo")

import numpy as np
import ml_dtypes

import concourse.bass as bass  # noqa: F401  (import keeps bass registered)
import concourse.tile as tile
from concourse import bacc, mybir
from concourse.bass_utils import run_bass_kernel_spmd

F32 = mybir.dt.float32
F32R = mybir.dt.float32r
BF16 = mybir.dt.bfloat16
AF = mybir.ActivationFunctionType
ALU = mybir.AluOpType

B, L_FULL, F = 4, 8192, 1024
H, D = 16, 64
N_CORES = 8
HG = H // 2  # heads per core = 8
HD = HG * D  # 512
EPS = 1e-6


def build_nc(L=L_FULL, TQ=512, TK=128):
    NKT = L // TK
    NQT = L // TQ
    FA = F // 128  # 8 f-tiles
    NM = HD // 128  # 4 hd-tiles

    nc = bacc.Bacc("TRN2", target_bir_lowering=False, debug=False)

    # inputs pre-tiled on host: [partition, tile, f-chunk, token]
    xqT = nc.dram_tensor("xqT", [128, NQT, FA, TQ], BF16, kind="ExternalInput")
    xkvT = nc.dram_tensor("xkvT", [128, NKT, FA, TK], BF16, kind="ExternalInput")
    wq = nc.dram_tensor("wq", [F, HD], BF16, kind="ExternalInput")
    wk = nc.dram_tensor("wk", [F, HD], BF16, kind="ExternalInput")
    wv = nc.dram_tensor("wv", [F, HD], BF16, kind="ExternalInput")
    wo = nc.dram_tensor("wo", [HD, F], BF16, kind="ExternalInput")
    em = nc.dram_tensor("ematrix", [HG, NM, 128], BF16, kind="ExternalInput")
    idm = nc.dram_tensor("ident", [128, 128], F32R, kind="ExternalInput")
    yT = nc.dram_tensor("yT", [128, NQT, FA, TQ], BF16, kind="ExternalOutput")

    wq_r = wq.rearrange("(a p) n -> p a n", p=128)
    wk_r = wk.rearrange("(a p) n -> p a n", p=128)
    wv_r = wv.rearrange("(a p) n -> p a n", p=128)
    wo_r = wo.rearrange("(m p) f -> p m f", p=128)

    with tile.TileContext(nc) as tc:
        with (
            tc.tile_pool(name="singles", bufs=1) as singles,
            tc.tile_pool(name="kv_in", bufs=3) as kv_in,
            tc.tile_pool(name="kwork", bufs=2) as kwork,
            tc.tile_pool(name="q_in", bufs=3) as q_in,
            tc.tile_pool(name="qwork", bufs=2) as qwork,
            tc.tile_pool(name="yout", bufs=2) as yout,
        ):
            ps_state_ctx = tc.tile_pool(name="ps_state", bufs=1, space="PSUM")
            ps_state = ps_state_ctx.__enter__()
            # ---- weights on the scalar + gpsimd rings (sync ring stays a
            # pure xkv/xq/y stream); first wk chunk small so the first k
            # matmul can start ~10us in
            wq_sb = singles.tile([128, FA, HD], BF16)
            wk_sb = singles.tile([128, FA, HD], BF16)
            wv_sb = singles.tile([128, FA, HD], BF16)
            wo_sb = singles.tile([128, NM, F], BF16)
            nc.scalar.dma_start(out=wk_sb[:, 0:2, :], in_=wk_r[:, 0:2, :])
            nc.scalar.dma_start(out=wk_sb[:, 2:4, :], in_=wk_r[:, 2:4, :])
            nc.gpsimd.dma_start(out=wk_sb[:, 4:6, :], in_=wk_r[:, 4:6, :])
            nc.gpsimd.dma_start(out=wk_sb[:, 6:8, :], in_=wk_r[:, 6:8, :])
            nc.gpsimd.dma_start(out=wv_sb[:, 4:6, :], in_=wv_r[:, 4:6, :])
            nc.gpsimd.dma_start(out=wv_sb[:, 6:8, :], in_=wv_r[:, 6:8, :])
            # persistent bf16 ones for the ksum column of the state matmul
            ones_sb = singles.tile([128, NM, 1], BF16)
            nc.vector.memset(ones_sb[:], 1.0)

            # persistent state accumulators, one PSUM bank per head pair j.
            # Single matmul per pair: lhsT = kp[:, pair d-range] (128 wide),
            # rhs = [v_h0 | v_h1 | 1] (129 wide). Rows 0:64 x cols 0:64 give
            # head 2j's kv, rows 64:128 x cols 64:128 head 2j+1's kv (cross
            # blocks never read); col 128 is the stacked ksum pair.
            st_ps = [
                ps_state.tile([128, 2 * D + 1], F32, tag=f"st{j}", name=f"st_ps{j}")
                for j in range(NM)
            ]

            # ---- phase 1: keys/values ----
            ps_kv_ctx = tc.tile_pool(name="ps_kv", bufs=2, space="PSUM")
            ps_kv = ps_kv_ctx.__enter__()
            # Software pipeline, k-side LAG ahead of v-side: the first k
            # projections only need Wk; the static PE instruction order must
            # have them first or the PE stalls on the v-side wait at startup.
            LAG = 2
            xq_pre = {}
            xkv_tiles = {}
            kp_tiles = {}

            def k_side(kt):
                xkv_t = kv_in.tile([128, FA, TK], BF16, tag="xkv", name=f"xkv{kt}")
                nc.sync.dma_start(out=xkv_t[:], in_=xkvT[:, kt, :, :])
                xkv_tiles[kt] = xkv_t
                pk = ps_kv.tile([128, HD], F32, tag="pk", name=f"pk{kt}")
                for a in range(FA):
                    nc.tensor.matmul(
                        pk[:],
                        lhsT=xkv_t[:, a, :],
                        rhs=wk_sb[:, a, :],
                        start=(a == 0),
                        stop=(a == FA - 1),
                    )
                # phi(k) = exp(min(k,0)) + max(k,0); kp in bf16 (state-only)
                tmp = kwork.tile([128, HD], F32, tag="tmp", name=f"tmp{kt}")
                nc.vector.tensor_scalar_min(tmp[:], pk[:], 0.0)
                ek = kwork.tile([128, HD], F32, tag="ek", name=f"ek{kt}")
                nc.scalar.activation(ek[:], tmp[:], AF.Exp)
                kp = kwork.tile([128, HD], BF16, tag="kp", name=f"kp{kt}", bufs=3)
                nc.vector.scalar_tensor_tensor(
                    kp[:], in0=pk[:], scalar=0.0, in1=ek[:], op0=ALU.max, op1=ALU.add
                )
                kp_tiles[kt] = kp

            def v_side(kt):
                xkv_t = xkv_tiles.pop(kt)
                kp = kp_tiles.pop(kt)
                pv = ps_kv.tile([128, HD], F32, tag="pv", name=f"pv{kt}")
                for a in range(FA):
                    nc.tensor.matmul(
                        pv[:],
                        lhsT=xkv_t[:, a, :],
                        rhs=wv_sb[:, a, :],
                        start=(a == 0),
                        stop=(a == FA - 1),
                    )
                # v pairs with trailing ones column: [v_h0 | v_h1 | 1]
                v_sb = kwork.tile([128, NM, 2 * D + 1], BF16, tag="v", name=f"v{kt}")
                nc.scalar.copy(
                    out=v_sb[:, :, 0 : 2 * D],
                    in_=pv[:].rearrange("p (j w) -> p j w", j=NM),
                )
                nc.vector.tensor_copy(v_sb[:, :, 2 * D : 2 * D + 1], ones_sb[:])
                # state accumulation, one stream per head pair bank
                for j in range(NM):
                    nc.tensor.matmul(
                        st_ps[j][:],
                        lhsT=kp[:, 2 * j * D : (2 * j + 2) * D],
                        rhs=v_sb[:, j, :],
                        start=(kt == 0),
                        stop=(kt == NKT - 1),
                    )

            # prologue: k-sides of the first LAG tiles run before any v work
            # (only needs Wk); wv chunks 0-3 issued on the sync ring between
            # the first xkv tiles.
            k_side(0)
            nc.sync.dma_start(out=wv_sb[:, 0:2, :], in_=wv_r[:, 0:2, :])
            k_side(1)
            nc.sync.dma_start(out=wv_sb[:, 2:4, :], in_=wv_r[:, 2:4, :])
            for i in range(LAG):
                v_side(i)
            for i in range(LAG, NKT):
                if i == NKT // 2:
                    # phase-2 weights + broadcast/identity mats on idle ring
                    for c in range(4):
                        nc.gpsimd.dma_start(
                            out=wq_sb[:, 2 * c : 2 * c + 2, :],
                            in_=wq_r[:, 2 * c : 2 * c + 2, :],
                        )
                    for m in range(NM):
                        nc.gpsimd.dma_start(out=wo_sb[:, m, :], in_=wo_r[:, m, :])
                if NKT > 16 and i in (NKT - 8, NKT - 4):
                    qi = 0 if i == NKT - 8 else 1
                    t_pre = q_in.tile([128, FA, TQ], BF16, tag="xq", name=f"xq_pre{qi}")
                    nc.sync.dma_start(out=t_pre[:], in_=xqT[:, qi, :, :])
                    xq_pre[qi] = t_pre
                k_side(i)
                v_side(i)

            ps_kv_ctx.__exit__(None, None, None)

            # broadcast matrix E: E[2j, j, 0:64] = 1, E[2j+1, j, 64:128] = 1
            # and 128x128 identity for the PE transpose
            e_sb = singles.tile([HG, NM, 128], BF16)
            nc.sync.dma_start(out=e_sb[:], in_=em[:])
            id_sb = singles.tile([128, 128], F32R)
            nc.sync.dma_start(out=id_sb[:], in_=idm[:])

            # ---- transition part A (emitted before phase-2 round 0 so the
            # vector engine fills kv2/blk while the PE runs q-projections):
            # block-diagonal kv pairs + block-diag ksum from the state PSUM.
            kv2 = singles.tile([128, NM, 128], F32R)
            nc.vector.memset(kv2[:].bitcast(F32), 0.0)
            blk = singles.tile([128, NM, HG], BF16)
            nc.vector.memset(blk[:], 0.0)
            for j in range(NM):
                nc.vector.tensor_copy(kv2[0:64, j, 0:D], st_ps[j][0:64, 0:D])
                nc.vector.tensor_copy(
                    kv2[64:128, j, D:128], st_ps[j][64:128, D : 2 * D]
                )
                nc.vector.tensor_copy(
                    blk[0:64, j, 2 * j : 2 * j + 1], st_ps[j][0:64, 2 * D : 2 * D + 1]
                )
                nc.vector.tensor_copy(
                    blk[64:128, j, 2 * j + 1 : 2 * j + 2],
                    st_ps[j][64:128, 2 * D : 2 * D + 1],
                )
            ps_state_ctx.__exit__(None, None, None)

            # phase-2 PSUM: pq/pd 2 + zb 2 + py 4 = 8 banks
            ps_q_ctx = tc.tile_pool(name="ps_q", bufs=2, space="PSUM")
            ps_q = ps_q_ctx.__enter__()

            kv2T = singles.tile([128, NM, 128], BF16)
            g_sb = singles.tile([128, NM, F], BF16)

            qp_tiles = {}
            z_tiles = {}

            def q_side(qt, emit_z):
                if qt in xq_pre:
                    xq_t = xq_pre.pop(qt)
                else:
                    xq_t = q_in.tile([128, FA, TQ], BF16, tag="xq", name=f"xq_t{qt}")
                    nc.sync.dma_start(out=xq_t[:], in_=xqT[:, qt, :, :])
                qp = qwork.tile([128, NM, TQ], BF16, tag="qp")
                for m in range(NM):
                    pq = ps_q.tile([128, TQ], F32, tag="pq")
                    for a in range(FA):
                        nc.tensor.matmul(
                            pq[:],
                            lhsT=wq_sb[:, a, m * 128 : (m + 1) * 128],
                            rhs=xq_t[:, a, :],
                            start=(a == 0),
                            stop=(a == FA - 1),
                        )
                    tmp2 = qwork.tile([128, TQ], F32, tag="tmp2")
                    nc.vector.tensor_scalar_min(tmp2[:], pq[:], 0.0)
                    eq = qwork.tile([128, TQ], F32, tag="eq")
                    nc.scalar.activation(eq[:], tmp2[:], AF.Exp)
                    nc.vector.scalar_tensor_tensor(
                        qp[:, m, :],
                        in0=pq[:],
                        scalar=0.0,
                        in1=eq[:],
                        op0=ALU.max,
                        op1=ALU.add,
                    )
                qp_tiles[qt] = qp
                # denominator [HG, TQ], accumulated over m; shares the pq
                # bank ring so steady-state PSUM stays at 8 banks
                pd = ps_q.tile([HG, TQ], F32, tag="pq", name=f"pd{qt}")
                for m in range(NM):
                    nc.tensor.matmul(
                        pd[:],
                        lhsT=blk[:, m, :],
                        rhs=qp[:, m, :],
                        start=(m == 0),
                        stop=(m == NM - 1),
                    )
                if emit_z:
                    z_recip(qt, pd)
                else:
                    pd_tiles[qt] = pd

            pd_tiles = {}

            def z_recip(qt, pd):
                zs = qwork.tile([HG, TQ], F32, tag="zs")
                nc.vector.tensor_scalar_add(zs[:], pd[:], EPS)
                z_sb = qwork.tile([HG, TQ], BF16, tag="z")
                with nc.allow_low_precision(reason="z rounds to bf16"):
                    nc.vector.reciprocal(z_sb[:], zs[:])
                z_tiles[qt] = z_sb

            def out_side(qt, next_qt):
                qp = qp_tiles.pop(qt)
                z_sb = z_tiles.pop(qt)
                qp2 = qwork.tile([128, NM, TQ], BF16, tag="qp2")
                for m in range(NM):
                    pzb = ps_zb.tile([128, TQ], F32, tag="zb", name=f"zb{qt}_{m}")
                    nc.tensor.matmul(
                        pzb[:], lhsT=e_sb[:, m, :], rhs=z_sb[:], start=True, stop=True
                    )
                    nc.vector.tensor_mul(qp2[:, m, :], qp[:, m, :], pzb[:])
                # z for the next round (pd already accumulated there)
                if next_qt is not None:
                    z_recip(next_qt, pd_tiles.pop(next_qt))
                # output projection: fo-quarters, m-outer accumulation so the
                # first y matmul only needs qp2[m=0]
                y_sb = yout.tile([128, FA, TQ], BF16)
                for q4 in range(4):
                    pys = [
                        ps_y.tile([128, TQ], F32, tag="py", name=f"py{qt}_{q4}_{w}")
                        for w in range(2)
                    ]
                    for m in range(NM):
                        for w in range(2):
                            fo = q4 * 2 + w
                            nc.tensor.matmul(
                                pys[w][:],
                                lhsT=g_sb[:, m, fo * 128 : (fo + 1) * 128],
                                rhs=qp2[:, m, :],
                                start=(m == 0),
                                stop=(m == NM - 1),
                            )
                    for w in range(2):
                        nc.scalar.copy(out=y_sb[:, q4 * 2 + w, :], in_=pys[w][:])
                nc.sync.dma_start(
                    out=yT[:, qt, 0 : FA // 2, :], in_=y_sb[:, 0 : FA // 2, :]
                )
                nc.sync.dma_start(
                    out=yT[:, qt, FA // 2 : FA, :], in_=y_sb[:, FA // 2 : FA, :]
                )

            # round 0: q-side, then transition part B (PE transpose + G) so
            # the state->Wo fold hides under the first q-projections.
            q_side(0, emit_z=True)
            ps_tr_ctx = tc.tile_pool(name="ps_tr", bufs=1, space="PSUM")
            ps_tr = ps_tr_ctx.__enter__()
            ps_g_ctx = tc.tile_pool(name="ps_g", bufs=1, space="PSUM")
            ps_g = ps_g_ctx.__enter__()
            for j in range(NM):
                trp = ps_tr.tile([128, 128], F32R, tag="tr", name=f"tr{j}")
                nc.tensor.transpose(trp[:], kv2[:, j, :], id_sb[:])
                nc.vector.tensor_copy(kv2T[:, j, :], trp[:].bitcast(F32))
            for j in range(NM):
                for fh in range(2):
                    gp = ps_g.tile([128, F // 2], F32, tag="g", name=f"g{j}_{fh}")
                    nc.tensor.matmul(
                        gp[:],
                        lhsT=kv2T[:, j, :],
                        rhs=wo_sb[:, j, fh * (F // 2) : (fh + 1) * (F // 2)],
                        start=True,
                        stop=True,
                    )
                    nc.scalar.copy(
                        out=g_sb[:, j, fh * (F // 2) : (fh + 1) * (F // 2)], in_=gp[:]
                    )
            ps_g_ctx.__exit__(None, None, None)
            ps_tr_ctx.__exit__(None, None, None)

            ps_zb_ctx = tc.tile_pool(name="ps_zb", bufs=2, space="PSUM")
            ps_zb = ps_zb_ctx.__enter__()
            ps_y_ctx = tc.tile_pool(name="ps_y", bufs=4, space="PSUM")
            ps_y = ps_y_ctx.__enter__()

            for qt in range(1, NQT):
                q_side(qt, emit_z=False)
                out_side(qt - 1, qt)
            out_side(NQT - 1, None)

            ps_y_ctx.__exit__(None, None, None)
            ps_zb_ctx.__exit__(None, None, None)
            ps_q_ctx.__exit__(None, None, None)

    nc.finalize()
    return nc


_NC_CACHE = {}


def _get_nc(L):
    if L not in _NC_CACHE:
        _NC_CACHE[L] = build_nc(L=L)
    return _NC_CACHE[L]


def _tile_T(x, nt, tt):
    # [L, F] f32 -> [128, nt, FA, tt] bf16 tile-contiguous transposed layout
    l_, f_ = x.shape
    fa = f_ // 128
    arr = np.ascontiguousarray(x.T).astype(ml_dtypes.bfloat16)
    return np.ascontiguousarray(
        arr.reshape(fa, 128, nt, tt).transpose(1, 2, 0, 3)
    )


def make_in_maps(inputs_q, inputs_kv, Wq, Wk, Wv, Wo, TQ=512, TK=128):
    inputs_q = np.asarray(inputs_q, dtype=np.float32)
    inputs_kv = np.asarray(inputs_kv, dtype=np.float32)
    Wq = np.asarray(Wq, dtype=np.float32)
    Wk = np.asarray(Wk, dtype=np.float32)
    Wv = np.asarray(Wv, dtype=np.float32)
    Wo = np.asarray(Wo, dtype=np.float32)
    b_ = inputs_q.shape[0]
    l_ = inputs_q.shape[1]
    nqt, nkt = l_ // TQ, l_ // TK
    xqT = [_tile_T(inputs_q[b], nqt, TQ) for b in range(b_)]
    xkvT = [_tile_T(inputs_kv[b], nkt, TK) for b in range(b_)]
    f_ = Wq.shape[0]
    BF = ml_dtypes.bfloat16
    wq_g = [
        np.ascontiguousarray(Wq[:, g * HG : (g + 1) * HG, :].reshape(f_, HD)).astype(BF)
        for g in range(2)
    ]
    wk_g = [
        np.ascontiguousarray(Wk[:, g * HG : (g + 1) * HG, :].reshape(f_, HD)).astype(BF)
        for g in range(2)
    ]
    wv_g = [
        np.ascontiguousarray(Wv[:, g * HG : (g + 1) * HG, :].reshape(f_, HD)).astype(BF)
        for g in range(2)
    ]
    wo_g = [
        np.ascontiguousarray(Wo[g * HG : (g + 1) * HG].reshape(HD, f_)).astype(BF)
        for g in range(2)
    ]
    em = make_ematrix()
    ident = np.eye(128, dtype=np.float32)
    in_maps = []
    for c in range(2 * b_):
        b, g = c // 2, c % 2
        in_maps.append(
            {
                "xqT": xqT[b],
                "xkvT": xkvT[b],
                "wq": wq_g[g],
                "wk": wk_g[g],
                "wv": wv_g[g],
                "wo": wo_g[g],
                "ematrix": em,
                "ident": ident,
            }
        )
    return in_maps


def make_ematrix():
    em = np.zeros((HG, HD // 128, 128), dtype=ml_dtypes.bfloat16)
    for j in range(HD // 128):
        em[2 * j, j, 0:64] = 1.0
        em[2 * j + 1, j, 64:128] = 1.0
    return em


def run(inputs_q, inputs_kv, Wq, Wk, Wv, Wo, trace=False, **spmd_kwargs):
    l_ = np.asarray(inputs_q).shape[1]
    TQ = 512
    nc = _get_nc(l_)
    in_maps = make_in_maps(inputs_q, inputs_kv, Wq, Wk, Wv, Wo)
    res = run_bass_kernel_spmd(
        nc, in_maps, list(range(len(in_maps))), trace=trace, **spmd_kwargs
    )
    b_ = len(in_maps) // 2
    nqt = l_ // TQ
    out = np.empty((b_, l_, F), dtype=np.float32)
    for b in range(b_):
        # yT: [128, NQT, FA, TQ] bf16 partials; f = a*128 + p, l = qt*TQ + t
        s = res.results[2 * b]["yT"].astype(np.float32) + res.results[2 * b + 1][
            "yT"
        ].astype(np.float32)
        np.copyto(out[b], s.transpose(1, 3, 2, 0).reshape(l_, F))
    return out, res


def kernel(inputs_q, inputs_kv, Wq, Wk, Wv, Wo):
    out, _ = run(inputs_q, inputs_kv, Wq, Wk, Wv, Wo)
    return out


# revision 34
# speedup vs baseline: 1.0565x; 1.0189x over previous
"""Multi-head linear attention on 8 Trainium2 NeuronCores.

Sharding: data-parallel over batch (4) x tensor-parallel over heads (2 groups
of 8). Core c handles batch c//2, head-group c%2. Each core computes its
head-group's partial output projection; the host sums the two partials per
batch.

All matmul operands are bf16 (f32 PSUM accumulation); inputs/weights are cast
to bf16 on the host and laid out tile-contiguously so every tile is one DMA
with 2KB+ lines. Per-core math (F=1024, L=8192, HG=8 heads, D=64, HD=512):

  phase 1 (keys/values, per 128-token tile):
    k = xkv @ Wk ; v = xkv @ Wv          ([tok, hd] in PSUM)
    kp = phi(k) = exp(min(k,0)) + max(k,0)   (bf16)
    st_j += kp_j^T @ [v_j | 1]           (PSUM state per head pair: kv + ksum)
  transition:
    kv2 = blockdiag(S_h) ; blk = blockdiag(ksum_h)
    kv2T = PE-transpose(kv2) ; G = kv2T^T @ Wo = S @ Wo   (folds Wo into state)
  phase 2 (queries, per 512-token tile, software-pipelined one stage):
    qT = Wq^T @ xq^T ; qp = phi(q)       ([hd, tok])
    den = blk^T @ qp ; z = 1/(den+eps)
    zb = E^T z (broadcast z over d) ; qp' = qp * zb
    yT += G^T @ qp'                      ([f, tok] partial, summed on host)
"""

import sys

sys.path.insert(0, "/opt/trn_rl_repo")

import numpy as np
import ml_dtypes

import concourse.bass as bass  # noqa: F401  (import keeps bass registered)
import concourse.tile as tile
from concourse import bacc, mybir
from concourse.bass_utils import run_bass_kernel_spmd

F32 = mybir.dt.float32
F32R = mybir.dt.float32r
BF16 = mybir.dt.bfloat16
AF = mybir.ActivationFunctionType
ALU = mybir.AluOpType

B, L_FULL, F = 4, 8192, 1024
H, D = 16, 64
N_CORES = 8
HG = H // 2  # heads per core = 8
HD = HG * D  # 512
EPS = 1e-6


def build_nc(L=L_FULL, TQ=512, TK=128):
    NKT = L // TK
    NQT = L // TQ
    FA = F // 128  # 8 f-tiles
    NM = HD // 128  # 4 hd-tiles

    nc = bacc.Bacc("TRN2", target_bir_lowering=False, debug=False)

    # inputs pre-tiled on host: [partition, tile, f-chunk, token]
    xqT = nc.dram_tensor("xqT", [128, NQT, FA, TQ], BF16, kind="ExternalInput")
    xkvT = nc.dram_tensor("xkvT", [128, NKT, FA, TK], BF16, kind="ExternalInput")
    wq = nc.dram_tensor("wq", [F, HD], BF16, kind="ExternalInput")
    wk = nc.dram_tensor("wk", [F, HD], BF16, kind="ExternalInput")
    wv = nc.dram_tensor("wv", [F, HD], BF16, kind="ExternalInput")
    wo = nc.dram_tensor("wo", [HD, F], BF16, kind="ExternalInput")
    em = nc.dram_tensor("ematrix", [HG, NM, 128], BF16, kind="ExternalInput")
    idm = nc.dram_tensor("ident", [128, 128], F32R, kind="ExternalInput")
    yT = nc.dram_tensor("yT", [128, NQT, FA, TQ], BF16, kind="ExternalOutput")

    wq_r = wq.rearrange("(a p) n -> p a n", p=128)
    wk_r = wk.rearrange("(a p) n -> p a n", p=128)
    wv_r = wv.rearrange("(a p) n -> p a n", p=128)
    wo_r = wo.rearrange("(m p) f -> p m f", p=128)

    with tile.TileContext(nc) as tc:
        with (
            tc.tile_pool(name="singles", bufs=1) as singles,
            tc.tile_pool(name="kv_in", bufs=3) as kv_in,
            tc.tile_pool(name="kwork", bufs=2) as kwork,
            tc.tile_pool(name="q_in", bufs=3) as q_in,
            tc.tile_pool(name="qwork", bufs=2) as qwork,
            tc.tile_pool(name="yout", bufs=2) as yout,
        ):
            # ps_kv below ps_state in the pool stack: at the transition,
            # ps_state exits first (LIFO) and ps_q lands on the freed pk/pv
            # banks, whose readers finish immediately - round 0's q matmuls
            # never wait for the state copy-out.
            ps_kv_ctx = tc.tile_pool(name="ps_kv", bufs=2, space="PSUM")
            ps_kv = ps_kv_ctx.__enter__()
            ps_state_ctx = tc.tile_pool(name="ps_state", bufs=1, space="PSUM")
            ps_state = ps_state_ctx.__enter__()
            # ---- weights on the scalar + gpsimd rings (sync ring stays a
            # pure xkv/xq/y stream); first wk chunk small so the first k
            # matmul can start ~10us in
            wq_sb = singles.tile([128, FA, HD], BF16)
            wk_sb = singles.tile([128, FA, HD], BF16)
            wv_sb = singles.tile([128, FA, HD], BF16)
            wo_sb = singles.tile([128, NM, F], BF16)
            nc.scalar.dma_start(out=wk_sb[:, 0:2, :], in_=wk_r[:, 0:2, :])
            nc.scalar.dma_start(out=wk_sb[:, 2:4, :], in_=wk_r[:, 2:4, :])
            nc.gpsimd.dma_start(out=wk_sb[:, 4:6, :], in_=wk_r[:, 4:6, :])
            nc.gpsimd.dma_start(out=wk_sb[:, 6:8, :], in_=wk_r[:, 6:8, :])
            nc.gpsimd.dma_start(out=wv_sb[:, 4:6, :], in_=wv_r[:, 4:6, :])
            nc.gpsimd.dma_start(out=wv_sb[:, 6:8, :], in_=wv_r[:, 6:8, :])
            # persistent bf16 ones for the ksum column of the state matmul
            ones_sb = singles.tile([128, NM, 1], BF16)
            nc.vector.memset(ones_sb[:], 1.0)

            # persistent state accumulators, one PSUM bank per head pair j.
            # Single matmul per pair: lhsT = kp[:, pair d-range] (128 wide),
            # rhs = [v_h0 | v_h1 | 1] (129 wide). Rows 0:64 x cols 0:64 give
            # head 2j's kv, rows 64:128 x cols 64:128 head 2j+1's kv (cross
            # blocks never read); col 128 is the stacked ksum pair.
            st_ps = [
                ps_state.tile([128, 2 * D + 1], F32, tag=f"st{j}", name=f"st_ps{j}")
                for j in range(NM)
            ]

            # ---- phase 1: keys/values ----
            # Software pipeline, k-side LAG ahead of v-side: the first k
            # projections only need Wk; the static PE instruction order must
            # have them first or the PE stalls on the v-side wait at startup.
            LAG = 2
            xq_pre = {}
            xkv_tiles = {}
            kp_tiles = {}

            def k_side(kt):
                xkv_t = kv_in.tile([128, FA, TK], BF16, tag="xkv", name=f"xkv{kt}")
                nc.sync.dma_start(out=xkv_t[:], in_=xkvT[:, kt, :, :])
                xkv_tiles[kt] = xkv_t
                pk = ps_kv.tile([128, HD], F32, tag="pk", name=f"pk{kt}")
                for a in range(FA):
                    nc.tensor.matmul(
                        pk[:],
                        lhsT=xkv_t[:, a, :],
                        rhs=wk_sb[:, a, :],
                        start=(a == 0),
                        stop=(a == FA - 1),
                    )
                # phi(k) = exp(min(k,0)) + max(k,0); kp in bf16 (state-only)
                tmp = kwork.tile([128, HD], F32, tag="tmp", name=f"tmp{kt}")
                nc.vector.tensor_scalar_min(tmp[:], pk[:], 0.0)
                ek = kwork.tile([128, HD], F32, tag="ek", name=f"ek{kt}")
                nc.scalar.activation(ek[:], tmp[:], AF.Exp)
                kp = kwork.tile([128, HD], BF16, tag="kp", name=f"kp{kt}", bufs=3)
                nc.vector.scalar_tensor_tensor(
                    kp[:], in0=pk[:], scalar=0.0, in1=ek[:], op0=ALU.max, op1=ALU.add
                )
                kp_tiles[kt] = kp

            st_pending = {}

            def state_mms(kt):
                kp, v_sb = st_pending.pop(kt)
                for j in range(NM):
                    nc.tensor.matmul(
                        st_ps[j][:],
                        lhsT=kp[:, 2 * j * D : (2 * j + 2) * D],
                        rhs=v_sb[:, j, :],
                        start=(kt == 0),
                        stop=(kt == NKT - 1),
                    )

            def v_side(kt):
                xkv_t = xkv_tiles.pop(kt)
                kp = kp_tiles.pop(kt)
                pv = ps_kv.tile([128, HD], F32, tag="pv", name=f"pv{kt}")
                for a in range(FA):
                    nc.tensor.matmul(
                        pv[:],
                        lhsT=xkv_t[:, a, :],
                        rhs=wv_sb[:, a, :],
                        start=(a == 0),
                        stop=(a == FA - 1),
                    )
                # v pairs with trailing ones column: [v_h0 | v_h1 | 1]
                v_sb = kwork.tile(
                    [128, NM, 2 * D + 1], BF16, tag="v", name=f"v{kt}", bufs=3
                )
                nc.scalar.copy(
                    out=v_sb[:, :, 0 : 2 * D],
                    in_=pv[:].rearrange("p (j w) -> p j w", j=NM),
                )
                nc.vector.tensor_copy(v_sb[:, :, 2 * D : 2 * D + 1], ones_sb[:])
                # state accumulation lags one tile: by the time state(kt-1)
                # issues, its kp has been ready for a full tile period and
                # the PE never waits on the phi chain
                st_pending[kt] = (kp, v_sb)
                if kt > 0:
                    state_mms(kt - 1)

            # prologue: k-sides of the first LAG tiles run before any v work
            # (only needs Wk); wv chunks 0-3 issued on the sync ring between
            # the first xkv tiles.
            k_side(0)
            nc.sync.dma_start(out=wv_sb[:, 0:2, :], in_=wv_r[:, 0:2, :])
            k_side(1)
            nc.sync.dma_start(out=wv_sb[:, 2:4, :], in_=wv_r[:, 2:4, :])
            for i in range(LAG):
                v_side(i)
            for i in range(LAG, NKT):
                if i == NKT // 2:
                    # phase-2 weights + broadcast/identity mats on idle ring
                    for c in range(4):
                        nc.gpsimd.dma_start(
                            out=wq_sb[:, 2 * c : 2 * c + 2, :],
                            in_=wq_r[:, 2 * c : 2 * c + 2, :],
                        )
                    for m in range(NM):
                        nc.gpsimd.dma_start(out=wo_sb[:, m, :], in_=wo_r[:, m, :])
                if NKT > 16 and i in (NKT - 8, NKT - 4):
                    qi = 0 if i == NKT - 8 else 1
                    t_pre = q_in.tile([128, FA, TQ], BF16, tag="xq", name=f"xq_pre{qi}")
                    nc.sync.dma_start(out=t_pre[:], in_=xqT[:, qi, :, :])
                    xq_pre[qi] = t_pre
                k_side(i)
                v_side(i)
            state_mms(NKT - 1)

            # broadcast matrix E: E[2j, j, 0:64] = 1, E[2j+1, j, 64:128] = 1
            # and 128x128 identity for the PE transpose
            e_sb = singles.tile([HG, NM, 128], BF16)
            nc.sync.dma_start(out=e_sb[:], in_=em[:])
            id_sb = singles.tile([128, 128], F32R)
            nc.sync.dma_start(out=id_sb[:], in_=idm[:])

            # ---- transition part A (emitted before phase-2 round 0 so the
            # vector engine fills kv2/blk while the PE runs q-projections):
            # block-diagonal kv pairs + block-diag ksum from the state PSUM.
            kv2 = singles.tile([128, NM, 128], F32R)
            nc.vector.memset(kv2[:].bitcast(F32), 0.0)
            blk = singles.tile([128, NM, HG], BF16)
            nc.vector.memset(blk[:], 0.0)
            for j in range(NM):
                nc.vector.tensor_copy(kv2[0:64, j, 0:D], st_ps[j][0:64, 0:D])
                nc.vector.tensor_copy(
                    kv2[64:128, j, D:128], st_ps[j][64:128, D : 2 * D]
                )
                nc.vector.tensor_copy(
                    blk[0:64, j, 2 * j : 2 * j + 1], st_ps[j][0:64, 2 * D : 2 * D + 1]
                )
                nc.vector.tensor_copy(
                    blk[64:128, j, 2 * j + 1 : 2 * j + 2],
                    st_ps[j][64:128, 2 * D : 2 * D + 1],
                )
            ps_state_ctx.__exit__(None, None, None)
            ps_kv_ctx.__exit__(None, None, None)

            # phase-2 PSUM: pq/pd 2 + zb 2 + py 4 = 8 banks
            ps_q_ctx = tc.tile_pool(name="ps_q", bufs=2, space="PSUM")
            ps_q = ps_q_ctx.__enter__()

            kv2T = singles.tile([128, NM, 128], BF16)
            g_sb = singles.tile([128, NM, F], BF16)

            qp_tiles = {}
            z_tiles = {}

            def q_side(qt, emit_z):
                if qt in xq_pre:
                    xq_t = xq_pre.pop(qt)
                else:
                    xq_t = q_in.tile([128, FA, TQ], BF16, tag="xq", name=f"xq_t{qt}")
                    nc.sync.dma_start(out=xq_t[:], in_=xqT[:, qt, :, :])
                qp = qwork.tile([128, NM, TQ], BF16, tag="qp")
                for m in range(NM):
                    pq = ps_q.tile([128, TQ], F32, tag="pq")
                    for a in range(FA):
                        nc.tensor.matmul(
                            pq[:],
                            lhsT=wq_sb[:, a, m * 128 : (m + 1) * 128],
                            rhs=xq_t[:, a, :],
                            start=(a == 0),
                            stop=(a == FA - 1),
                        )
                    tmp2 = qwork.tile([128, TQ], F32, tag="tmp2")
                    nc.vector.tensor_scalar_min(tmp2[:], pq[:], 0.0)
                    eq = qwork.tile([128, TQ], F32, tag="eq")
                    nc.scalar.activation(eq[:], tmp2[:], AF.Exp)
                    nc.vector.scalar_tensor_tensor(
                        qp[:, m, :],
                        in0=pq[:],
                        scalar=0.0,
                        in1=eq[:],
                        op0=ALU.max,
                        op1=ALU.add,
                    )
                qp_tiles[qt] = qp
                # denominator [HG, TQ], accumulated over m; shares the pq
                # bank ring so steady-state PSUM stays at 8 banks
                pd = ps_q.tile([HG, TQ], F32, tag="pq", name=f"pd{qt}")
                for m in range(NM):
                    nc.tensor.matmul(
                        pd[:],
                        lhsT=blk[:, m, :],
                        rhs=qp[:, m, :],
                        start=(m == 0),
                        stop=(m == NM - 1),
                    )
                if emit_z:
                    z_recip(qt, pd)
                else:
                    pd_tiles[qt] = pd

            pd_tiles = {}

            def z_recip(qt, pd):
                zs = qwork.tile([HG, TQ], F32, tag="zs")
                nc.vector.tensor_scalar_add(zs[:], pd[:], EPS)
                z_sb = qwork.tile([HG, TQ], BF16, tag="z")
                with nc.allow_low_precision(reason="z rounds to bf16"):
                    nc.vector.reciprocal(z_sb[:], zs[:])
                z_tiles[qt] = z_sb

            def out_side(qt, next_qt):
                qp = qp_tiles.pop(qt)
                z_sb = z_tiles.pop(qt)
                qp2 = qwork.tile([128, NM, TQ], BF16, tag="qp2")
                for m in range(NM):
                    pzb = ps_zb.tile([128, TQ], F32, tag="zb", name=f"zb{qt}_{m}")
                    nc.tensor.matmul(
                        pzb[:], lhsT=e_sb[:, m, :], rhs=z_sb[:], start=True, stop=True
                    )
                    nc.vector.tensor_mul(qp2[:, m, :], qp[:, m, :], pzb[:])
                # z for the next round (pd already accumulated there)
                if next_qt is not None:
                    z_recip(next_qt, pd_tiles.pop(next_qt))
                # output projection: fo-quarters, m-outer accumulation so the
                # first y matmul only needs qp2[m=0]
                y_sb = yout.tile([128, FA, TQ], BF16)
                for q4 in range(4):
                    pys = [
                        ps_y.tile([128, TQ], F32, tag="py", name=f"py{qt}_{q4}_{w}")
                        for w in range(2)
                    ]
                    for m in range(NM):
                        for w in range(2):
                            fo = q4 * 2 + w
                            nc.tensor.matmul(
                                pys[w][:],
                                lhsT=g_sb[:, m, fo * 128 : (fo + 1) * 128],
                                rhs=qp2[:, m, :],
                                start=(m == 0),
                                stop=(m == NM - 1),
                            )
                    for w in range(2):
                        nc.scalar.copy(out=y_sb[:, q4 * 2 + w, :], in_=pys[w][:])
                    if qt == NQT - 1:
                        # drain the tail: ship each quarter as soon as copied
                        nc.sync.dma_start(
                            out=yT[:, qt, 2 * q4 : 2 * q4 + 2, :],
                            in_=y_sb[:, 2 * q4 : 2 * q4 + 2, :],
                        )
                if qt < NQT - 1:
                    nc.sync.dma_start(
                        out=yT[:, qt, 0 : FA // 2, :], in_=y_sb[:, 0 : FA // 2, :]
                    )
                    nc.sync.dma_start(
                        out=yT[:, qt, FA // 2 : FA, :], in_=y_sb[:, FA // 2 : FA, :]
                    )

            # round 0: q-side, then transition part B (PE transpose + G) so
            # the state->Wo fold hides under the first q-projections.
            q_side(0, emit_z=True)
            ps_tr_ctx = tc.tile_pool(name="ps_tr", bufs=1, space="PSUM")
            ps_tr = ps_tr_ctx.__enter__()
            ps_g_ctx = tc.tile_pool(name="ps_g", bufs=1, space="PSUM")
            ps_g = ps_g_ctx.__enter__()
            for j in range(NM):
                trp = ps_tr.tile([128, 128], F32R, tag="tr", name=f"tr{j}")
                nc.tensor.transpose(trp[:], kv2[:, j, :], id_sb[:])
                nc.vector.tensor_copy(kv2T[:, j, :], trp[:].bitcast(F32))
            for j in range(NM):
                for fh in range(2):
                    gp = ps_g.tile([128, F // 2], F32, tag="g", name=f"g{j}_{fh}")
                    nc.tensor.matmul(
                        gp[:],
                        lhsT=kv2T[:, j, :],
                        rhs=wo_sb[:, j, fh * (F // 2) : (fh + 1) * (F // 2)],
                        start=True,
                        stop=True,
                    )
                    nc.scalar.copy(
                        out=g_sb[:, j, fh * (F // 2) : (fh + 1) * (F // 2)], in_=gp[:]
                    )
            ps_g_ctx.__exit__(None, None, None)
            ps_tr_ctx.__exit__(None, None, None)

            ps_zb_ctx = tc.tile_pool(name="ps_zb", bufs=2, space="PSUM")
            ps_zb = ps_zb_ctx.__enter__()
            ps_y_ctx = tc.tile_pool(name="ps_y", bufs=4, space="PSUM")
            ps_y = ps_y_ctx.__enter__()

            for qt in range(1, NQT):
                q_side(qt, emit_z=False)
                out_side(qt - 1, qt)
            out_side(NQT - 1, None)

            ps_y_ctx.__exit__(None, None, None)
            ps_zb_ctx.__exit__(None, None, None)
            ps_q_ctx.__exit__(None, None, None)

    nc.finalize()
    return nc


_NC_CACHE = {}


def _get_nc(L):
    if L not in _NC_CACHE:
        _NC_CACHE[L] = build_nc(L=L)
    return _NC_CACHE[L]


def _tile_T(x, nt, tt):
    # [L, F] f32 -> [128, nt, FA, tt] bf16 tile-contiguous transposed layout
    l_, f_ = x.shape
    fa = f_ // 128
    arr = np.ascontiguousarray(x.T).astype(ml_dtypes.bfloat16)
    return np.ascontiguousarray(
        arr.reshape(fa, 128, nt, tt).transpose(1, 2, 0, 3)
    )


def make_in_maps(inputs_q, inputs_kv, Wq, Wk, Wv, Wo, TQ=512, TK=128):
    inputs_q = np.asarray(inputs_q, dtype=np.float32)
    inputs_kv = np.asarray(inputs_kv, dtype=np.float32)
    Wq = np.asarray(Wq, dtype=np.float32)
    Wk = np.asarray(Wk, dtype=np.float32)
    Wv = np.asarray(Wv, dtype=np.float32)
    Wo = np.asarray(Wo, dtype=np.float32)
    b_ = inputs_q.shape[0]
    l_ = inputs_q.shape[1]
    nqt, nkt = l_ // TQ, l_ // TK
    xqT = [_tile_T(inputs_q[b], nqt, TQ) for b in range(b_)]
    xkvT = [_tile_T(inputs_kv[b], nkt, TK) for b in range(b_)]
    f_ = Wq.shape[0]
    BF = ml_dtypes.bfloat16
    wq_g = [
        np.ascontiguousarray(Wq[:, g * HG : (g + 1) * HG, :].reshape(f_, HD)).astype(BF)
        for g in range(2)
    ]
    wk_g = [
        np.ascontiguousarray(Wk[:, g * HG : (g + 1) * HG, :].reshape(f_, HD)).astype(BF)
        for g in range(2)
    ]
    wv_g = [
        np.ascontiguousarray(Wv[:, g * HG : (g + 1) * HG, :].reshape(f_, HD)).astype(BF)
        for g in range(2)
    ]
    wo_g = [
        np.ascontiguousarray(Wo[g * HG : (g + 1) * HG].reshape(HD, f_)).astype(BF)
        for g in range(2)
    ]
    em = make_ematrix()
    ident = np.eye(128, dtype=np.float32)
    in_maps = []
    for c in range(2 * b_):
        b, g = c // 2, c % 2
        in_maps.append(
            {
                "xqT": xqT[b],
                "xkvT": xkvT[b],
                "wq": wq_g[g],
                "wk": wk_g[g],
                "wv": wv_g[g],
                "wo": wo_g[g],
                "ematrix": em,
                "ident": ident,
            }
        )
    return in_maps


def make_ematrix():
    em = np.zeros((HG, HD // 128, 128), dtype=ml_dtypes.bfloat16)
    for j in range(HD // 128):
        em[2 * j, j, 0:64] = 1.0
        em[2 * j + 1, j, 64:128] = 1.0
    return em


def run(inputs_q, inputs_kv, Wq, Wk, Wv, Wo, trace=False, **spmd_kwargs):
    l_ = np.asarray(inputs_q).shape[1]
    TQ = 512
    nc = _get_nc(l_)
    in_maps = make_in_maps(inputs_q, inputs_kv, Wq, Wk, Wv, Wo)
    res = run_bass_kernel_spmd(
        nc, in_maps, list(range(len(in_maps))), trace=trace, **spmd_kwargs
    )
    b_ = len(in_maps) // 2
    nqt = l_ // TQ
    out = np.empty((b_, l_, F), dtype=np.float32)
    for b in range(b_):
        # yT: [128, NQT, FA, TQ] bf16 partials; f = a*128 + p, l = qt*TQ + t
        s = res.results[2 * b]["yT"].astype(np.float32) + res.results[2 * b + 1][
            "yT"
        ].astype(np.float32)
        np.copyto(out[b], s.transpose(1, 3, 2, 0).reshape(l_, F))
    return out, res


def kernel(inputs_q, inputs_kv, Wq, Wk, Wv, Wo):
    out, _ = run(inputs_q, inputs_kv, Wq, Wk, Wv, Wo)
    return out
